# revision 3
# baseline (speedup 1.0000x reference)
"""Bidirectional GRU (nn_CustomGRU) Trainium2 Bass kernel.

Problem: S=512, B=128, I=H=1024, bidirectional GRU, fp32.
  out_f = GRU_f(x),  out_b = GRU_b(x[::-1])  (backward outputs NOT re-flipped)
  output = concat([out_f, out_b], axis=2)  -> [S, B, 2H]

Sharding: 8 cores = 2 direction groups x 4-way batch shard (B_local=32).
Each core independently runs one direction's GRU on its batch slice.

Per 16-step block (single fused loop):
  1. gi = Wih @ x (+ biases) for the block, feature-major, N=512 bf16
     matmuls, result kept in SBUF (no DRAM round-trip).
  2. 16 recurrence steps. Whh-stationary bf16 matmuls into PSUM; the
     contraction k-loop is split in half (k0-3 reads hbf_lo, k4-7 reads
     hbf_hi) so the gate chain of the low half of h overlaps the high-half
     matmuls and the next step's k0-3 matmuls overlap the high-half gate
     chain -> PE stays dense (and HAM-warm).
     PSUM accumulation: one start=True / stop=True per bank per step;
     interleaved groups rely on per-element has_written semantics.

Layouts (per core):
  x_fm    [KO=8, 128, S, BL]   bf16   feature-major input (i = 128*ko + ki)
  wih_t   [KO=8, 128, 3H]      bf16   Wih.T  (wih_t[ko, ki, m] = Wih[m, 128*ko+ki])
  whh_t   [KO=8, 128, 3H]      bf16   Whh.T
  gi_bias [128, 24]            fp32   bih + bhh (r,z rows only), [p, c] = vec[128c+p]
  bhh_n   [128, 8]             fp32   bhh n-gate rows
  out_h   [8, 128, S, BL]      fp32   h history, (c, p) = hidden channel 128c+p
"""

import os

import numpy as np
import ml_dtypes

import concourse.bass as bass
import concourse.mybir as mybir
import concourse.tile as tile
from concourse import bacc
from concourse.bass import ds
from concourse.bass_utils import run_bass_kernel_spmd

S, B, I, H = 512, 128, 1024, 1024
NCORES = 8
BL = B // 4          # batch per core (4-way shard x 2 directions)
KO = I // 128        # 8 contraction chunks
MC = (3 * H) // 128  # 24 gate-row chunks (r: 0-7, z: 8-15, n: 16-23)
HC = H // 128        # 8 hidden-channel chunks
TBLK = 16            # timesteps per block

BF16 = mybir.dt.bfloat16
F32 = mybir.dt.float32
AF = mybir.ActivationFunctionType


def build_program(seq_len=S, bl=BL, tblk=TBLK):
    nc = bacc.Bacc(
        "TRN2",
        target_bir_lowering=False,
        debug=False,
        enable_asserts=False,
        num_devices=NCORES,
    )

    x_d = nc.dram_tensor("x_fm", [KO, 128, seq_len, bl], BF16, kind="ExternalInput")
    wih_d = nc.dram_tensor("wih_t", [KO, 128, 3 * H], BF16, kind="ExternalInput")
    whh_d = nc.dram_tensor("whh_t", [KO, 128, 3 * H], BF16, kind="ExternalInput")
    gibias_d = nc.dram_tensor("gi_bias", [128, MC], F32, kind="ExternalInput")
    bhhn_d = nc.dram_tensor("bhh_n", [128, HC], F32, kind="ExternalInput")
    out_d = nc.dram_tensor("out_h", [HC, 128, seq_len, bl], F32, kind="ExternalOutput")

    with tile.TileContext(nc) as tc:
        with tc.tile_pool(name="static", bufs=1) as spool, \
             tc.tile_pool(name="xp", bufs=2) as xpool, \
             tc.tile_pool(name="gip", bufs=1) as gipool, \
             tc.tile_pool(name="hist", bufs=2) as histpool, \
             tc.tile_pool(name="tmp", bufs=2) as tmppool, \
             tc.tile_pool(name="gps", bufs=3, space="PSUM") as gps, \
             tc.tile_pool(name="rps", bufs=2, space="PSUM") as rps:
            wih_sb = spool.tile([128, KO, 3 * H], BF16)
            nc.sync.dma_start(wih_sb, wih_d[:].rearrange("ko ki m -> ki ko m"))
            whh_sb = spool.tile([128, KO, 3 * H], BF16)
            nc.sync.dma_start(whh_sb, whh_d[:].rearrange("ko ki m -> ki ko m"))
            gibias_sb = spool.tile([128, MC], F32)
            nc.sync.dma_start(gibias_sb, gibias_d[:])
            bhhn_sb = spool.tile([128, HC], F32)
            nc.sync.dma_start(bhhn_sb, bhhn_d[:])
            # persistent recurrent state, split into low/high halves of H
            h32 = spool.tile([128, HC, bl], F32)
            hbf_lo = spool.tile([128, 4, bl], BF16)
            hbf_hi = spool.tile([128, 4, bl], BF16)
            nc.vector.memset(h32, 0.0)
            nc.vector.memset(hbf_lo, 0.0)
            nc.vector.memset(hbf_hi, 0.0)

            # gate order within each phase: r first (needed earliest),
            # z last (its chain is the step tail). (psum idx base, m base)
            GATE_ORDER = ((0, 0), (8, 2 * H), (4, H))  # r, n, z

            with tc.For_i(0, seq_len, tblk) as s0:
                # ---- input-gate matmuls for this block (kept in SBUF) ----
                x_blk = xpool.tile([128, KO, tblk, bl], BF16)
                nc.sync.dma_start(
                    x_blk,
                    x_d[:, :, ds(s0, tblk), :].rearrange("ko ki s b -> ki ko s b"),
                )
                gi_sb = gipool.tile([128, MC, tblk, bl], BF16)
                for c in range(MC):
                    ps = gps.tile([128, tblk, bl], F32)
                    for k in range(KO):
                        nc.tensor.matmul(
                            ps,
                            wih_sb[:, k, c * 128:(c + 1) * 128],
                            x_blk[:, k],
                            start=(k == 0),
                            stop=(k == KO - 1),
                        )
                    nc.scalar.activation(
                        gi_sb[:, c], ps, AF.Identity,
                        bias=gibias_sb[:, c:c + 1], scale=1.0,
                    )

                # ---- recurrence ----
                hist = histpool.tile([128, HC, tblk, bl], F32)
                for t in range(tblk):
                    ps_h = [rps.tile([128, 12, bl], F32, tag=f"ps{h}",
                                     name=f"ps{h}")
                            for h in range(2)]
                    # phase A: k 0..3 (reads hbf_lo only)
                    for half in range(2):
                        first = True
                        for (pbase, mbase) in GATE_ORDER:
                            for ci in range(4):
                                m0 = mbase + (half * 4 + ci) * 128
                                for k in range(4):
                                    nc.tensor.matmul(
                                        ps_h[half][:, pbase + ci],
                                        whh_sb[:, k, m0:m0 + 128],
                                        hbf_lo[:, k],
                                        start=first, stop=False,
                                        skip_group_check=True,
                                    )
                                    first = False
                    # phase B: k 4..7 (reads hbf_hi), gates per half
                    for half in range(2):
                        for gidx, (pbase, mbase) in enumerate(GATE_ORDER):
                            for ci in range(4):
                                m0 = mbase + (half * 4 + ci) * 128
                                for k in range(4, 8):
                                    last = (gidx == 2 and ci == 3 and k == 7)
                                    nc.tensor.matmul(
                                        ps_h[half][:, pbase + ci],
                                        whh_sb[:, k, m0:m0 + 128],
                                        hbf_hi[:, k - 4],
                                        start=False, stop=last,
                                        skip_group_check=True,
                                    )
                        # gates for this half
                        ps = ps_h[half]
                        sl = slice(half * 4, half * 4 + 4)
                        g_r = gi_sb[:, half * 4:half * 4 + 4, t]
                        g_z = gi_sb[:, 8 + half * 4:12 + half * 4, t]
                        g_n = gi_sb[:, 16 + half * 4:20 + half * 4, t]
                        rpre = tmppool.tile([128, 4, bl], F32, tag=f"rpre{half}")
                        nc.vector.tensor_add(rpre, ps[:, 0:4], g_r)
                        r_t = tmppool.tile([128, 4, bl], F32, tag=f"r{half}")
                        nc.scalar.activation(r_t, rpre, AF.Sigmoid)
                        hn = tmppool.tile([128, 4, bl], F32, tag=f"hn{half}")
                        nc.vector.tensor_tensor(
                            hn, ps[:, 8:12],
                            bhhn_sb[:, sl, None].to_broadcast((128, 4, bl)),
                            mybir.AluOpType.add,
                        )
                        rn = tmppool.tile([128, 4, bl], F32, tag=f"rn{half}")
                        nc.vector.tensor_mul(rn, hn, r_t)
                        npre = tmppool.tile([128, 4, bl], F32, tag=f"npre{half}")
                        nc.vector.tensor_add(npre, rn, g_n)
                        ntile = tmppool.tile([128, 4, bl], F32, tag=f"n{half}")
                        nc.scalar.activation(ntile, npre, AF.Tanh)
                        zpre = tmppool.tile([128, 4, bl], F32, tag=f"zpre{half}")
                        nc.vector.tensor_add(zpre, ps[:, 4:8], g_z)
                        zs = tmppool.tile([128, 4, bl], F32, tag=f"z{half}")
                        nc.scalar.activation(zs, zpre, AF.Sigmoid)
                        prev = (h32[:, sl] if t == 0 else hist[:, sl, t - 1])
                        dtile = tmppool.tile([128, 4, bl], F32, tag=f"d{half}")
                        nc.vector.tensor_sub(dtile, prev, ntile)
                        zd = tmppool.tile([128, 4, bl], F32, tag=f"zd{half}")
                        nc.vector.tensor_mul(zd, dtile, zs)
                        hbf_half = hbf_lo if half == 0 else hbf_hi
                        nc.vector.tensor_add(hbf_half, ntile, zd)
                        nc.vector.tensor_add(hist[:, sl, t], ntile, zd)
                nc.vector.tensor_copy(h32, hist[:, :, tblk - 1])
                nc.sync.dma_start(
                    out_d[:, :, ds(s0, tblk), :].rearrange("c ki s b -> ki c s b"),
                    hist,
                )

    nc.compile()
    return nc


def _prep_weights(Wih, Whh, bih, bhh):
    wih_t = np.ascontiguousarray(Wih.T.reshape(KO, 128, 3 * H)).astype(ml_dtypes.bfloat16)
    whh_t = np.ascontiguousarray(Whh.T.reshape(KO, 128, 3 * H)).astype(ml_dtypes.bfloat16)
    gib = bih.astype(np.float64).copy()
    gib[:2 * H] += bhh[:2 * H].astype(np.float64)
    gi_bias = np.ascontiguousarray(gib.reshape(MC, 128).T).astype(np.float32)
    bhh_n = np.ascontiguousarray(bhh[2 * H:].reshape(HC, 128).T).astype(np.float32)
    return wih_t, whh_t, gi_bias, bhh_n


def _prep_x(x_slice):
    # x_slice: [S, BL, I] fp32 -> [KO, 128, S, BL] bf16 feature-major
    xt = np.ascontiguousarray(x_slice.transpose(2, 0, 1))  # [I, S, BL]
    return xt.reshape(KO, 128, x_slice.shape[0], x_slice.shape[1]).astype(ml_dtypes.bfloat16)


_prog_cache = {}


def _get_program():
    key = (S, BL, TBLK)
    if key not in _prog_cache:
        _prog_cache[key] = build_program()
    return _prog_cache[key]


def kernel(inpt, Wih_f, Whh_f, bih_f, bhh_f, Wih_b, Whh_b, bih_b, bhh_b):
    inpt = np.asarray(inpt, dtype=np.float32)
    nc = _get_program()

    wf = _prep_weights(np.asarray(Wih_f), np.asarray(Whh_f),
                       np.asarray(bih_f), np.asarray(bhh_f))
    wb = _prep_weights(np.asarray(Wih_b), np.asarray(Whh_b),
                       np.asarray(bih_b), np.asarray(bhh_b))
    x_rev = inpt[::-1]

    in_maps = []
    for core in range(NCORES):
        direction = core // 4
        b0 = (core % 4) * BL
        w = wf if direction == 0 else wb
        xs = (inpt if direction == 0 else x_rev)[:, b0:b0 + BL, :]
        in_maps.append({
            "x_fm": _prep_x(xs),
            "wih_t": w[0], "whh_t": w[1], "gi_bias": w[2], "bhh_n": w[3],
        })

    trace = bool(int(os.environ.get("GRU_TRACE", "0")))
    res = run_bass_kernel_spmd(
        nc, in_maps, core_ids=list(range(NCORES)), trace=trace,
    )
    if trace and res.exec_time_ns is not None:
        print(f"HW exec time: {res.exec_time_ns} ns")
        if res.instructions_and_trace is not None:
            print(f"Trace: {res.instructions_and_trace[1]}")

    out = np.empty((S, B, 2 * H), dtype=np.float32)
    for core in range(NCORES):
        direction = core // 4
        b0 = (core % 4) * BL
        oc = res.results[core]["out_h"]  # [HC, 128, S, BL]
        out[:, b0:b0 + BL, direction * H:(direction + 1) * H] = (
            oc.transpose(2, 3, 0, 1).reshape(S, BL, H)
        )
    return out
